# revision 1
# baseline (speedup 1.0000x reference)
"""AncProbsLayer Trainium2 kernel.

Computes anc[b, l, k*26+c] = P[b,k,token(b,l),c] where P[b,k] =
expm(tau_b * Q_k), via:
  host:   Q_k -> symmetrized eigendecomposition (reversible rate matrix)
          Q = D^-1/2 V Lam V^T D^1/2,  tau = softplus(tau_kernel)[rate_indices]
  device: e = exp(tau_b * lam_k)          (ACT)
          P[b,k] = U_k @ (diag(e) W_k)    (PE, fp32)
          per-b lookup table T[b] (26 x 208) in SBUF (bf16)
          out rows = onehot26(token) @ T[b]   (PE, bf16 -> fp32 PSUM)
Pure data-parallel over batch: 64 of 512 sequences per NeuronCore.
"""

import sys
import numpy as np

for _p in ("/opt/trn_rl_repo",):
    if _p not in sys.path:
        sys.path.insert(0, _p)

import ml_dtypes
import concourse.bass as bass
import concourse.tile as tile
from concourse import mybir
from concourse.bass_utils import run_bass_kernel_spmd
from concourse.vector_clock import ScopedClock

B, L, K, NR, S = 512, 1024, 8, 512, 20
EXT = 26
ROW = K * EXT          # 208 output row width
N_CORES = 8
B_SH = B // N_CORES    # 64 sequences per core


def _patch_tile_drain():
    """This container's walrus rejects >1 sync-wait per instruction.  Split
    extra waits onto no-op instructions inserted just before, on the same
    engine (same program order on that engine => identical semantics)."""
    if getattr(tile.TileContext, "_drain_patched", False):
        return

    orig_lower = tile.TileContext._lower_ordered_insts

    def _split_lower(self, ordered):
        nc = self.nc
        for bb_name, insts in list(ordered.items()):
            new = []
            for inst in insts:
                si = getattr(inst, "sync_info", None)
                if si is not None and len(si.on_wait) > 1:
                    waits = list(si.on_wait)
                    for w in waits[:-1]:
                        nop = mybir.InstNoOp(
                            name=nc.get_next_instruction_name(),
                            ins=[], outs=[],
                            sync_info=mybir.SyncInfo(on_wait=[w], on_update=[]),
                            bass_nofuse=True,
                            engine=inst.engine,
                        )
                        new.append(nop)
                    inst.sync_info = mybir.SyncInfo(
                        on_wait=[waits[-1]], on_update=list(si.on_update)
                    )
                new.append(inst)
            ordered[bb_name] = new
        return orig_lower(self, ordered)

    tile.TileContext._lower_ordered_insts = _split_lower

    def _drain_and_barrier(self, tick_clock, wait_clock):
        nc = self.nc
        drain_inst = nc.sync.drain()
        wait_clock.add_sem_waits(
            drain_inst.ins, ScopedClock({None: tick_clock.global_clock})
        )
        si = drain_inst.ins.sync_info
        if si is not None and len(si.on_wait) > 1:
            waits = list(si.on_wait)
            drain_inst.ins.sync_info = mybir.SyncInfo(
                on_wait=[waits[0]], on_update=list(si.on_update)
            )
            for w in waits[1:]:
                d2 = nc.sync.drain()
                d2.ins.sync_info = mybir.SyncInfo(on_wait=[w], on_update=[])
        nc.all_engine_barrier()
        assert self.sems is not None
        popped = nc._tile_sem_poison_stack.pop()
        assert popped is self._sem_poison
        nc.clear_and_free_semaphores(list(self.sems.allocated().values()))
        nc.all_engine_barrier()

    tile.TileContext._drain_and_barrier = _drain_and_barrier
    tile.TileContext._drain_patched = True


def _softplus(x):
    return np.log1p(np.exp(-np.abs(x))) + np.maximum(x, 0.0)


def _host_prep(tau_kernel, exchangeability_kernel, frequencies):
    """Tiny parameter preprocessing (K=8 20x20 matrices) in float64."""
    E = exchangeability_kernel.astype(np.float64)
    freq = frequencies.astype(np.float64)
    eye = np.eye(S)
    R = _softplus(0.5 * (E + np.swapaxes(E, -1, -2))) * (1.0 - eye)
    Q = R * freq[None, None, :]
    diag = Q.sum(-1, keepdims=True)
    Q = Q - diag * eye
    mue = (freq[None, :, None] * diag).sum(-2, keepdims=True)
    Q = Q / np.maximum(mue, 1e-16)

    d = np.sqrt(freq)
    Sym = d[None, :, None] * Q / d[None, None, :]
    Sym = 0.5 * (Sym + np.swapaxes(Sym, -1, -2))
    lam, V = np.linalg.eigh(Sym)                       # (K,S), (K,S,S)
    U = V / d[:, None][None]                           # D^-1/2 V
    W = np.swapaxes(V, -1, -2) * d[None, None, :]      # V^T D^1/2

    # device layouts, eig-index i on partitions; pack ut|w|lam row-wise
    ut = np.transpose(U, (2, 0, 1)).reshape(S, K * S)     # ut[i,(k,t)]=U_k[t,i]
    w = np.transpose(W, (1, 0, 2)).reshape(S, K * S)      # w[i,(k,c)]=W_k[i,c]
    uwl = np.concatenate([ut, w, lam.T], axis=1).astype(np.float32)

    tmpl = np.zeros((EXT, ROW), np.float32)
    for t in range(S, EXT):
        for k in range(K):
            tmpl[t, k * EXT + t] = 1.0
    tmpl = tmpl.astype(ml_dtypes.bfloat16)

    iota = (np.arange(128) % 32).astype(np.float32).reshape(128, 1)
    return uwl, tmpl, iota


def _build_bass():
    _patch_tile_drain()
    f32, bf16, u8 = mybir.dt.float32, mybir.dt.bfloat16, mybir.dt.uint8

    nc = bass.Bass("TRN2", target_bir_lowering=False, debug=False,
                   num_devices=N_CORES)
    tok_d = nc.declare_dram_parameter("tok", [B_SH, L], u8, isOutput=False)
    uwl_d = nc.declare_dram_parameter("uwl", [S, 2 * K * S + K + B_SH], f32,
                                      isOutput=False)
    tmpl_d = nc.declare_dram_parameter("tmpl", [EXT, ROW], bf16, isOutput=False)
    iota_d = nc.declare_dram_parameter("iota", [128, 1], f32, isOutput=False)
    out_d = nc.declare_dram_parameter("out", [B_SH, L, ROW], f32, isOutput=True)

    # batch chunks: table built + released to the gather stage per chunk so
    # output DMA starts before the whole table exists
    CHUNKS = ((0, 4), (4, 4), (8, 8), (16, 16), (32, 16), (48, 16))

    with tile.TileContext(nc) as tc:
        with (
            tc.tile_pool(name="consts", bufs=1) as consts,
            tc.tile_pool(name="tbl", bufs=1) as tbl,
            tc.tile_pool(name="small", bufs=3) as small,
            tc.tile_pool(name="ohp", bufs=3) as ohp,
            tc.tile_pool(name="stage", bufs=10) as stagep,
            tc.tile_pool(name="ps_expm", bufs=2, space="PSUM") as ps_expm,
            tc.tile_pool(name="ps_gat", bufs=6, space="PSUM") as ps_gat,
        ):
            # ---- constants (single DMA for ut|w|lam|tau-replicated) ----
            uwl_t = consts.tile([S, 2 * K * S + K + B_SH], f32)
            nc.sync.dma_start(out=uwl_t[:], in_=uwl_d[:, :])
            ut_t = uwl_t[:, 0 : K * S].rearrange("i (k t) -> i k t", k=K)
            w_t = uwl_t[:, K * S : 2 * K * S].rearrange("i (k t) -> i k t", k=K)
            lam_t = uwl_t[:, 2 * K * S : 2 * K * S + K]
            tau_rep = uwl_t[:, 2 * K * S + K : 2 * K * S + K + B_SH]
            iota_t = consts.tile([128, 1], f32)
            nc.sync.dma_start(out=iota_t[:], in_=iota_d[:, :])
            # PE pre-warm: dependency-free matmuls during the startup window
            # flip the HAM clock gate to 8/8 before the real matmuls arrive
            warm_in = consts.tile([128, 320], bf16)
            nc.gpsimd.memset(warm_in, 0)
            # dummy exp: pull the lazy ACT exp-table load off the critical
            # path (it costs ~4us at first use)
            warm_act = consts.tile([S, 1], f32)
            nc.scalar.activation(
                out=warm_act[:], in_=warm_in[0:S, 0:1],
                func=mybir.ActivationFunctionType.Exp,
            )
            for _ in range(12):
                wps = ps_expm.tile([S, 16, S], f32, tag="ps_expm")
                nc.tensor.matmul(
                    wps[:, :, :].rearrange("p a b -> p (a b)"),
                    lhsT=warm_in[:, 0:S], rhs=warm_in[:],
                    start=True, stop=True,
                )

            # ---- e = exp(tau * lam_k), one mult + one exp ----
            e_all = consts.tile([S, K, B_SH], f32)
            nc.vector.tensor_tensor(
                out=e_all[:],
                in0=lam_t.unsqueeze(2).broadcast_to([S, K, B_SH]),
                in1=tau_rep.unsqueeze(1).broadcast_to([S, K, B_SH]),
                op=mybir.AluOpType.mult,
            )
            nc.scalar.activation(
                out=e_all[:], in_=e_all[:],
                func=mybir.ActivationFunctionType.Exp,
            )

            # ---- per-chunk lookup tables at partition bases 0/32/64/96 ----
            tmpl_ap = tmpl_d[:, :]
            tok_ap = tok_d[:, :]

            def build_chunk(ci):
                c0, cn = CHUNKS[ci]
                T4 = tbl.tile([128, cn, ROW], bf16, tag=f"T4_{ci}")
                nc.gpsimd.dma_start(
                    out=T4[0:EXT, :, :],
                    in_=bass.AP(tensor=tmpl_ap.tensor, offset=0,
                                ap=[[ROW, EXT], [0, cn], [1, ROW]]),
                )
                # rhs for all K at once: one big DVE op amortizes the
                # per-op SBUF read-write bubble
                rhs = small.tile([S, K, 16, S], f32, tag="rhs")
                nc.vector.tensor_tensor(
                    out=rhs[:, :, 0:cn, :],
                    in0=w_t[:, :, :].unsqueeze(2).broadcast_to([S, K, cn, S]),
                    in1=e_all[:, :, c0 : c0 + cn].unsqueeze(3).broadcast_to(
                        [S, K, cn, S]
                    ),
                    op=mybir.AluOpType.mult,
                )
                for k in range(K):
                    pst = ps_expm.tile([S, 16, S], f32, tag="ps_expm")
                    nc.tensor.matmul(
                        pst[:, 0:cn, :], lhsT=ut_t[:, k, :],
                        rhs=rhs[:, k, 0:cn, :],
                        start=True, stop=True,
                    )
                    nc.any.tensor_copy(
                        out=T4[0:S, :, k * EXT : k * EXT + S],
                        in_=pst[:, 0:cn, :],
                    )
                # replicate table rows 0..25 to partition bases 32/64/96
                for rep in (32, 64, 96):
                    nc.gpsimd.dma_start(
                        out=T4[rep : rep + EXT, :, :], in_=T4[0:EXT, :, :]
                    )
                return T4

            def gather_group(g, T4, c0):
                # seq position l = p*8 + c (p = psum partition, c in 0..7) so
                # each partition's 8 output rows are contiguous in DRAM
                # tokens for 4 seqs, each replicated across a 32-partition
                # block (rows 26..31 junk, never matched nor read)
                tokr = ohp.tile([128, 128, 8], u8, tag="tokr")
                nc.gpsimd.dma_start(
                    out=tokr[:],
                    in_=bass.AP(tensor=tok_ap.tensor, offset=g * 4 * L,
                                ap=[[L, 4], [0, 32], [1, L]]),
                )
                oh = ohp.tile([128, 128, 8], bf16, tag="oh")
                nc.vector.tensor_scalar(
                    out=oh[:], in0=tokr[:], scalar1=iota_t[:], scalar2=None,
                    op0=mybir.AluOpType.is_equal,
                )
                for b4 in range(4):
                    b = g * 4 + b4
                    stage = stagep.tile([128, 8, ROW], f32, tag="stage")
                    for cp in range(4):
                        ps = ps_gat.tile([128, 2, ROW], f32, tag="ps_gat")
                        for h in range(2):
                            c = 2 * cp + h
                            nc.tensor.matmul(
                                ps[:, h, :],
                                lhsT=oh[b4 * 32 : b4 * 32 + EXT, :, c].squeeze(),
                                rhs=T4[b4 * 32 : b4 * 32 + EXT, b - c0, :],
                                start=True, stop=True,
                                tile_position=(b4 * 32, 0),
                            )
                        nc.any.tensor_copy(
                            out=stage[:, 2 * cp : 2 * cp + 2, :], in_=ps[:]
                        )
                    eng = nc.sync if b % 2 == 0 else nc.scalar
                    eng.dma_start(
                        out=out_d[b].rearrange("(p c) j -> p c j", p=128),
                        in_=stage[:],
                    )

            # chunk -> gather groups (4 seqs each)
            grp_of = []
            for ci, (c0, cn) in enumerate(CHUNKS):
                grp_of.append(range(c0 // 4, (c0 + cn) // 4))

            # software-pipelined emission: build chunk ci+1 right after the
            # groups of chunk ci (PE FIFO: gather matmuls of ci run before
            # the next chunk's expm matmuls)
            tchunks = {0: build_chunk(0), 1: None}
            tchunks[1] = build_chunk(1)
            for ci in range(len(CHUNKS)):
                for g in grp_of[ci]:
                    gather_group(g, tchunks[ci], CHUNKS[ci][0])
                if ci + 2 < len(CHUNKS):
                    tchunks[ci + 2] = build_chunk(ci + 2)
    return nc


_NC_CACHE = None


def kernel(inputs, rate_indices, tau_kernel, exchangeability_kernel,
           frequencies):
    global _NC_CACHE
    tok = np.asarray(inputs, dtype=np.uint8)
    tau = _softplus(np.asarray(tau_kernel, dtype=np.float64))[
        np.asarray(rate_indices, dtype=np.int64)
    ].astype(np.float32)
    uwl, tmpl, iota = _host_prep(
        np.asarray(tau_kernel), np.asarray(exchangeability_kernel),
        np.asarray(frequencies),
    )

    if _NC_CACHE is None:
        _NC_CACHE = _build_bass()
    nc = _NC_CACHE

    in_maps = []
    for c in range(N_CORES):
        sl = slice(c * B_SH, (c + 1) * B_SH)
        in_maps.append({
            "tok": np.ascontiguousarray(tok[sl]),
            "uwl": np.ascontiguousarray(np.concatenate(
                [uwl, np.broadcast_to(tau[sl], (S, B_SH))], axis=1)),
            "tmpl": tmpl, "iota": iota,
        })
    res = run_bass_kernel_spmd(nc, in_maps, core_ids=list(range(N_CORES)))
    out = np.concatenate([res.results[c]["out"] for c in range(N_CORES)], axis=0)
    return out.astype(np.float32, copy=False)


if __name__ == "__main__":
    rng = np.random.default_rng(0)
    ins = {
        "inputs": rng.integers(0, EXT, size=(B, L)).astype(np.int32),
        "rate_indices": rng.integers(0, NR, size=(B,)).astype(np.int32),
        "tau_kernel": rng.standard_normal(NR).astype(np.float32),
        "exchangeability_kernel": rng.standard_normal((K, S, S)).astype(np.float32),
        "frequencies": rng.uniform(0.01, 1.0, S).astype(np.float32),
    }
    o = kernel(**ins)
    print("kernel out", o.shape, o.dtype)

